# revision 3
# baseline (speedup 1.0000x reference)
import sys

if "/opt/trn_rl_repo" not in sys.path:
    sys.path.insert(0, "/opt/trn_rl_repo")

import numpy as np

import concourse.bacc as bacc
import concourse.tile as tile
from concourse import bass_utils, mybir
from concourse.bass import ts
from concourse.masks import make_identity

F32 = mybir.dt.float32
F32R = mybir.dt.float32r
EXP = mybir.ActivationFunctionType.Exp

# Problem shapes (nn_MultiHeadedAttention): B=2, S=2048, D=1024, H=16, DH=64.
# Sharding: 16 heads over 8 cores (2 heads/core, 128 features). QKV is
# column-parallel, out-projection row-parallel; host sums the 8 partials.
B, S, D, H = 2, 2048, 1024, 16
DH = D // H
NC = 8
T = B * S                  # 4096 tokens
NCHUNK = T // 512          # 8 token chunks of 512
KCH = D // 128             # 8 contraction chunks
NJ = S // 128              # 16 key tiles per batch
QC = S // 512              # 4 query chunks per batch

_CACHE = {}


def _build():
    if "nc" in _CACHE:
        return _CACHE["nc"]

    nc = bacc.Bacc("TRN2", target_bir_lowering=False, debug=False,
                   enable_asserts=True, num_devices=NC)

    xT = nc.dram_tensor("xT", [D, T], F32, kind="ExternalInput").ap()
    wq = nc.dram_tensor("wq", [D, 128], F32, kind="ExternalInput").ap()
    wk = nc.dram_tensor("wk", [D, 128], F32, kind="ExternalInput").ap()
    wv = nc.dram_tensor("wv", [D, 128], F32, kind="ExternalInput").ap()
    wo = nc.dram_tensor("wo", [128, D], F32, kind="ExternalInput").ap()
    bq = nc.dram_tensor("bq", [128, 1], F32, kind="ExternalInput").ap()
    bk = nc.dram_tensor("bk", [128, 1], F32, kind="ExternalInput").ap()
    outT = nc.dram_tensor("outT", [D, T], F32, kind="ExternalOutput").ap()

    with tile.TileContext(nc) as tc:
        with (
            tc.tile_pool(name="wpool", bufs=1) as wpool,
            tc.tile_pool(name="qk", bufs=1) as qk_pool,
            tc.tile_pool(name="vtm", bufs=1) as vtm_pool,
            tc.tile_pool(name="on", bufs=1) as on_pool,
            tc.tile_pool(name="xin", bufs=12) as xin_pool,
            tc.tile_pool(name="vst", bufs=2) as vst_pool,
            tc.tile_pool(name="epool", bufs=4) as epool,
            tc.tile_pool(name="npool", bufs=4) as npool,
            tc.tile_pool(name="ostage", bufs=3) as ostage_pool,
        ):
            # ---- persistent weights / constants ----
            wq_sb = wpool.tile([128, D], F32R)
            wk_sb = wpool.tile([128, D], F32R)
            wv_sb = wpool.tile([128, D], F32R)
            wo_sb = wpool.tile([128, D], F32R)
            bq_sb = wpool.tile([128, 1], F32)
            bk_sb = wpool.tile([128, 1], F32)
            ident = wpool.tile([128, 128], F32)
            for k in range(KCH):
                nc.sync.dma_start(wq_sb[:, ts(k, 128)],
                                  wq[ts(k, 128), :].bitcast(F32R))
                nc.sync.dma_start(wk_sb[:, ts(k, 128)],
                                  wk[ts(k, 128), :].bitcast(F32R))
                nc.sync.dma_start(wv_sb[:, ts(k, 128)],
                                  wv[ts(k, 128), :].bitcast(F32R))
            nc.sync.dma_start(wo_sb[:], wo[:].bitcast(F32R))
            nc.sync.dma_start(bq_sb[:], bq[:])
            nc.sync.dma_start(bk_sb[:], bk[:])
            make_identity(nc, ident[:])

            # Warm the ACT exp table during phase 1.
            dummy = wpool.tile([1, 2], F32)
            nc.vector.memset(dummy[:], 0.0)
            nc.scalar.activation(dummy[:], dummy[:], EXP)

            # persistent activations
            qn = [qk_pool.tile([128, 512], F32R, name=f"qn{n}")
                  for n in range(NCHUNK)]
            kn = [qk_pool.tile([128, 512], F32R, name=f"kn{n}")
                  for n in range(NCHUNK)]
            on = [on_pool.tile([128, 512], F32R, name=f"on{n}")
                  for n in range(NCHUNK)]
            v_tm = {}
            for hh in range(2):
                for J in range(2 * NJ):
                    v_tm[(hh, J)] = vtm_pool.tile(
                        [128, 65], F32R, name=f"vtm{hh}_{J}")

            # ---- phase 1: projections + v transposes ----
            with (
                tc.tile_pool(name="ps_acc", bufs=6, space="PSUM") as ps_acc,
                tc.tile_pool(name="ps_t", bufs=2, space="PSUM") as ps_t,
            ):
                for n in range(NCHUNK):
                    xps = []
                    for k in range(KCH):
                        xp = xin_pool.tile([128, 512], F32R, tag="xp",
                                           name=f"xp{n}_{k}")
                        nc.sync.dma_start(
                            xp[:], xT[ts(k, 128), ts(n, 512)].bitcast(F32R))
                        xps.append(xp)
                    q_ps = ps_acc.tile([128, 512], F32, tag="acc",
                                       name=f"qps{n}")
                    k_ps = ps_acc.tile([128, 512], F32, tag="acc",
                                       name=f"kps{n}")
                    v_ps = ps_acc.tile([128, 512], F32, tag="acc",
                                       name=f"vps{n}")
                    for k in range(KCH):
                        fl = dict(start=(k == 0), stop=(k == KCH - 1))
                        nc.tensor.matmul(q_ps[:], wq_sb[:, ts(k, 128)],
                                         xps[k][:], **fl)
                        nc.tensor.matmul(k_ps[:], wk_sb[:, ts(k, 128)],
                                         xps[k][:], **fl)
                        nc.tensor.matmul(v_ps[:], wv_sb[:, ts(k, 128)],
                                         xps[k][:], **fl)
                    nc.vector.tensor_scalar_add(qn[n][:], q_ps[:], bq_sb[:])
                    nc.vector.tensor_scalar_add(kn[n][:], k_ps[:], bk_sb[:])
                    vst = vst_pool.tile([128, 512], F32, name=f"vst{n}")
                    nc.vector.tensor_copy(vst[:], v_ps[:])
                    for hh in range(2):
                        hs = slice(hh * 64, (hh + 1) * 64)
                        for jj in range(4):
                            J = 4 * n + jj
                            t_ps = ps_t.tile([128, 64], F32, tag="t",
                                             name=f"tps{n}_{hh}_{jj}")
                            nc.tensor.transpose(t_ps[:], vst[hs, ts(jj, 128)],
                                                ident[hs, hs])
                            vt = v_tm[(hh, J)]
                            nc.vector.tensor_copy(vt[:, 0:64], t_ps[:])
                            nc.vector.memset(vt[:, 64:65].bitcast(F32), 1.0)

            # ---- phase 2: attention + out-projection ----
            with (
                tc.tile_pool(name="ps_s", bufs=4, space="PSUM") as ps_s,
                tc.tile_pool(name="ps_o", bufs=2, space="PSUM") as ps_o,
                tc.tile_pool(name="ps_op", bufs=2, space="PSUM") as ps_op,
            ):
                for b in range(B):
                    for qc in range(QC):
                        n = b * QC + qc
                        o_ps = [ps_o.tile([65, 512], F32, tag="o",
                                          name=f"ops{n}_{hh}")
                                for hh in range(2)]
                        for j in range(NJ):
                            s_ps = []
                            for hh in range(2):
                                hs = slice(hh * 64, (hh + 1) * 64)
                                sp = ps_s.tile([128, 512], F32, tag="s",
                                               name=f"sps{n}_{j}_{hh}")
                                nc.tensor.matmul(
                                    sp[:],
                                    kn[b * QC + j // 4][hs, ts(j % 4, 128)],
                                    qn[n][hs, :], start=True, stop=True)
                                s_ps.append(sp)
                            for hh in range(2):
                                e_sb = epool.tile([128, 512], F32R, tag="e",
                                                  name=f"e{n}_{j}_{hh}")
                                nc.scalar.activation(e_sb[:], s_ps[hh][:], EXP)
                                nc.tensor.matmul(
                                    o_ps[hh][:], v_tm[(hh, b * NJ + j)][:],
                                    e_sb[:],
                                    start=(j == 0), stop=(j == NJ - 1))
                        for hh in range(2):
                            hs = slice(hh * 64, (hh + 1) * 64)
                            r_sb = npool.tile([1, 512], F32, tag="r",
                                              name=f"r{n}_{hh}")
                            nc.vector.reciprocal(r_sb[:], o_ps[hh][64:65, :])
                            rb_sb = npool.tile([64, 512], F32, tag="rb",
                                               name=f"rb{n}_{hh}")
                            nc.gpsimd.partition_broadcast(rb_sb[:], r_sb[:])
                            nc.vector.tensor_tensor(
                                out=on[n][hs, :], in0=o_ps[hh][0:64, :],
                                in1=rb_sb[:], op=mybir.AluOpType.mult)
                        for m in range(KCH):
                            op_ps = ps_op.tile([128, 512], F32, tag="op",
                                               name=f"opps{n}_{m}")
                            nc.tensor.matmul(op_ps[:], wo_sb[:, ts(m, 128)],
                                             on[n][:], start=True, stop=True)
                            ost = ostage_pool.tile([128, 512], F32, tag="ost",
                                                   name=f"ost{n}_{m}")
                            nc.vector.tensor_copy(ost[:], op_ps[:])
                            nc.sync.dma_start(outT[ts(m, 128), ts(n, 512)],
                                              ost[:])

    nc.compile()
    _CACHE["nc"] = nc
    return nc


def kernel(x, Wq, bq, Wk, bk, Wv, bv, Wo, bo):
    x = np.asarray(x, np.float32)
    Wq, bq = np.asarray(Wq, np.float32), np.asarray(bq, np.float32)
    Wk, bk = np.asarray(Wk, np.float32), np.asarray(bk, np.float32)
    Wv, bv = np.asarray(Wv, np.float32), np.asarray(bv, np.float32)
    Wo, bo = np.asarray(Wo, np.float32), np.asarray(bo, np.float32)

    nc = _build()

    xT = np.ascontiguousarray(x.reshape(T, D).T)
    scale = np.float32(1.0 / np.sqrt(DH))
    in_maps = []
    for c in range(NC):
        sl = slice(128 * c, 128 * (c + 1))
        in_maps.append({
            "xT": xT,
            "wq": np.ascontiguousarray((scale * Wq[sl, :]).T),
            "wk": np.ascontiguousarray(Wk[sl, :].T),
            "wv": np.ascontiguousarray(Wv[sl, :].T),
            "wo": np.ascontiguousarray(Wo[:, sl].T),
            "bq": np.ascontiguousarray((scale * bq[sl])[:, None]),
            "bk": np.ascontiguousarray(bk[sl][:, None]),
        })

    res = bass_utils.run_bass_kernel_spmd(nc, in_maps, core_ids=list(range(NC)))

    acc = np.zeros((D, T), np.float64)
    for c in range(NC):
        acc += res.results[c]["outT"]
    # v-bias folds through softmax (rows sum to 1): + bv @ Wo.T; plus bo.
    const = bo.astype(np.float64) + bv.astype(np.float64) @ Wo.T.astype(np.float64)
    out = acc.T + const[None, :]
    return out.astype(np.float32).reshape(B, S, D)


# revision 7
# speedup vs baseline: 1.2266x; 1.2266x over previous
import sys

if "/opt/trn_rl_repo" not in sys.path:
    sys.path.insert(0, "/opt/trn_rl_repo")

import numpy as np

import concourse.bacc as bacc
import concourse.tile as tile
from concourse import bass_utils, mybir
from concourse.bass import ts
from concourse.masks import make_identity

F32 = mybir.dt.float32
F32R = mybir.dt.float32r
EXP = mybir.ActivationFunctionType.Exp

# nn_MultiHeadedAttention: B=2, S=2048, D=1024, H=16, DH=64.
# 16 heads over 8 cores (2 heads/core = 128 features). QKV column-parallel,
# out-projection row-parallel, host sums the 8 partial outputs.
B, S, D, H = 2, 2048, 1024, 16
DH = D // H
NC = 8
T = B * S                  # 4096 tokens
NCHUNK = T // 512          # 8 token chunks of 512
KCH = D // 128             # 8 contraction chunks
NJ = S // 128              # 16 key tiles per batch
QC = S // 512              # 4 query chunks per batch

_CACHE = {}


def _build():
    if "nc" in _CACHE:
        return _CACHE["nc"]

    nc = bacc.Bacc("TRN2", target_bir_lowering=False, debug=False,
                   enable_asserts=True, num_devices=NC)

    xT = nc.dram_tensor("xT", [D, T], F32, kind="ExternalInput").ap()
    wq = nc.dram_tensor("wq", [D, 128], F32, kind="ExternalInput").ap()
    wk = nc.dram_tensor("wk", [D, 128], F32, kind="ExternalInput").ap()
    wv = nc.dram_tensor("wv", [D, 128], F32, kind="ExternalInput").ap()
    wo = nc.dram_tensor("wo", [128, D], F32, kind="ExternalInput").ap()
    bq = nc.dram_tensor("bq", [128, 1], F32, kind="ExternalInput").ap()
    bk = nc.dram_tensor("bk", [128, 1], F32, kind="ExternalInput").ap()
    outT = nc.dram_tensor("outT", [D, T], F32, kind="ExternalOutput").ap()

    with tile.TileContext(nc) as tc:
        with (
            tc.tile_pool(name="wpool", bufs=1) as wpool,
            tc.tile_pool(name="qk", bufs=1) as qk_pool,
            tc.tile_pool(name="vtm", bufs=1) as vtm_pool,
            tc.tile_pool(name="on", bufs=1) as on_pool,
            tc.tile_pool(name="xin", bufs=12) as xin_pool,
            tc.tile_pool(name="vst", bufs=2) as vst_pool,
            tc.tile_pool(name="epool", bufs=6) as epool,
            tc.tile_pool(name="npool", bufs=2) as npool,
            tc.tile_pool(name="ostage", bufs=3) as ostage_pool,
            # PSUM: psA 4 banks (qkv accum / scores), psO 2 (o accum),
            # psOP 2 (v transposes / out-projection)
            tc.tile_pool(name="psA", bufs=2, space="PSUM") as psA,
            tc.tile_pool(name="psO", bufs=1, space="PSUM") as psO,
            tc.tile_pool(name="psOP", bufs=2, space="PSUM") as psOP,
        ):
            # ---- persistent weights / constants ----
            wq_sb = wpool.tile([128, D], F32R)
            wk_sb = wpool.tile([128, D], F32R)
            wv_sb = wpool.tile([128, D], F32R)
            wo_sb = wpool.tile([128, D], F32R)
            bq_sb = wpool.tile([128, 1], F32)
            bk_sb = wpool.tile([128, 1], F32)
            ident = wpool.tile([128, 128], F32)
            for k in range(KCH):
                nc.sync.dma_start(wq_sb[:, ts(k, 128)],
                                  wq[ts(k, 128), :].bitcast(F32R))
                nc.sync.dma_start(wk_sb[:, ts(k, 128)],
                                  wk[ts(k, 128), :].bitcast(F32R))
                nc.sync.dma_start(wv_sb[:, ts(k, 128)],
                                  wv[ts(k, 128), :].bitcast(F32R))
            nc.sync.dma_start(wo_sb[:], wo[:].bitcast(F32R))
            nc.sync.dma_start(bq_sb[:], bq[:])
            nc.sync.dma_start(bk_sb[:], bk[:])
            make_identity(nc, ident[:])

            # Warm the ACT exp table while phase 1 runs.
            dummy = wpool.tile([1, 2], F32)
            nc.vector.memset(dummy[:], 0.0)
            nc.scalar.activation(dummy[:], dummy[:], EXP)

            # persistent activations
            qn = [qk_pool.tile([128, 512], F32R, name=f"qn{n}")
                  for n in range(NCHUNK)]
            kn = [qk_pool.tile([128, 512], F32R, name=f"kn{n}")
                  for n in range(NCHUNK)]
            on = [on_pool.tile([128, 512], F32R, name=f"on{n}")
                  for n in range(NCHUNK)]
            v_tm = {}
            for hh in range(2):
                for J in range(2 * NJ):
                    v_tm[(hh, J)] = vtm_pool.tile(
                        [128, 65], F32R, name=f"vtm{hh}_{J}")

            # ---- phase 1: projections + v transposes ----
            for n in range(NCHUNK):
                xps = []
                for k in range(KCH):
                    xp = xin_pool.tile([128, 512], F32R, tag="xp",
                                       name=f"xp{n}_{k}")
                    nc.sync.dma_start(
                        xp[:], xT[ts(k, 128), ts(n, 512)].bitcast(F32R))
                    xps.append(xp)
                qk_ps = psA.tile([128, 1024], F32, tag="A", name=f"qkps{n}")
                v_ps = psA.tile([128, 1024], F32, tag="A", name=f"vps{n}")
                for k in range(KCH):
                    nc.tensor.matmul(qk_ps[:, 0:512], wq_sb[:, ts(k, 128)],
                                     xps[k][:], start=(k == 0),
                                     stop=(k == KCH - 1))
                for k in range(KCH):
                    nc.tensor.matmul(qk_ps[:, 512:1024], wk_sb[:, ts(k, 128)],
                                     xps[k][:], start=(k == 0),
                                     stop=(k == KCH - 1))
                nc.vector.tensor_scalar_add(qn[n][:], qk_ps[:, 0:512],
                                            bq_sb[:])
                for k in range(KCH):
                    nc.tensor.matmul(v_ps[:, 0:512], wv_sb[:, ts(k, 128)],
                                     xps[k][:], start=(k == 0),
                                     stop=(k == KCH - 1))
                nc.vector.tensor_scalar_add(kn[n][:], qk_ps[:, 512:1024],
                                            bk_sb[:])
                vst = vst_pool.tile([128, 512], F32, name=f"vst{n}")
                nc.vector.tensor_copy(vst[:], v_ps[:, 0:512])
                for hh in range(2):
                    hs = slice(hh * 64, (hh + 1) * 64)
                    for jj in range(4):
                        J = 4 * n + jj
                        t_ps = psOP.tile([128, 512], F32, tag="OP",
                                         name=f"tps{n}_{hh}_{jj}")
                        nc.tensor.transpose(t_ps[:, 0:64],
                                            vst[hs, ts(jj, 128)],
                                            ident[hs, hs])
                        vt = v_tm[(hh, J)]
                        nc.vector.tensor_copy(vt[:, 0:64], t_ps[:, 0:64])
                        nc.vector.memset(vt[:, 64:65].bitcast(F32), 1.0)

            # ---- phase 2: attention + out-projection ----
            # Normalize + out-projection for a finished query chunk is
            # emitted a few j-iterations into the NEXT chunk so the PE
            # never waits on the normalization chain.
            def emit_norm_outproj(n, o_ps):
                oc = npool.tile([64, 1024], F32, tag="oc", name=f"oc{n}")
                nc.vector.tensor_copy(oc[:], o_ps[0:64, :])
                # sums row lives at partition 64; reciprocal_approx_fast
                # mishandles base_partition != 0, so shift to partition 0.
                sums_sb = npool.tile([1, 1024], F32, tag="sums",
                                     name=f"sums{n}")
                nc.vector.tensor_copy(sums_sb[:], o_ps[64:65, :])
                r_sb = npool.tile([1, 1024], F32, tag="r", name=f"r{n}")
                nc.vector.reciprocal_approx_fast(r_sb[:], sums_sb[0:1, :])
                for hh in range(2):
                    hs = slice(hh * 64, (hh + 1) * 64)
                    rb = npool.tile([64, 512], F32, tag=f"rb{hh}",
                                    name=f"rb{n}_{hh}")
                    nc.gpsimd.partition_broadcast(
                        rb[:], r_sb[0:1, ts(hh, 512)])
                    nc.vector.tensor_tensor(
                        out=on[n][hs, :], in0=oc[0:64, ts(hh, 512)],
                        in1=rb[:], op=mybir.AluOpType.mult)
                for m in range(KCH):
                    op_ps = psOP.tile([128, 512], F32, tag="OP",
                                      name=f"opps{n}_{m}")
                    nc.tensor.matmul(op_ps[:, 0:512], wo_sb[:, ts(m, 128)],
                                     on[n][:], start=True, stop=True)
                    ost = ostage_pool.tile([128, 512], F32, tag="ost",
                                           name=f"ost{n}_{m}")
                    nc.vector.tensor_copy(ost[:], op_ps[:, 0:512])
                    nc.sync.dma_start(outT[ts(m, 128), ts(n, 512)], ost[:])

            pending = None
            for b in range(B):
                for qc in range(QC):
                    n = b * QC + qc
                    o_ps = psO.tile([65, 1024], F32, tag="O", name=f"ops{n}")
                    e_prev = None
                    for j in range(NJ):
                        s_ps = psA.tile([128, 1024], F32, tag="A",
                                        name=f"sps{n}_{j}")
                        for hh in range(2):
                            hs = slice(hh * 64, (hh + 1) * 64)
                            nc.tensor.matmul(
                                s_ps[:, ts(hh, 512)],
                                kn[b * QC + j // 4][hs, ts(j % 4, 128)],
                                qn[n][hs, :], start=True, stop=True)
                        e_sb = epool.tile([128, 1024], F32R, tag="e",
                                          name=f"e{n}_{j}")
                        nc.scalar.activation(e_sb[:], s_ps[:], EXP)
                        if j >= 1:
                            for hh in range(2):
                                nc.tensor.matmul(
                                    o_ps[0:65, ts(hh, 512)],
                                    v_tm[(hh, b * NJ + j - 1)][:],
                                    e_prev[:, ts(hh, 512)],
                                    start=(j - 1 == 0), stop=False)
                        if j == 4 and pending is not None:
                            emit_norm_outproj(*pending)
                            pending = None
                        e_prev = e_sb
                    for hh in range(2):
                        nc.tensor.matmul(
                            o_ps[0:65, ts(hh, 512)],
                            v_tm[(hh, b * NJ + NJ - 1)][:],
                            e_prev[:, ts(hh, 512)],
                            start=False, stop=True)
                    pending = (n, o_ps)
            emit_norm_outproj(*pending)

    nc.compile()
    _CACHE["nc"] = nc
    return nc


def _prep_in_maps(x, Wq, bq, Wk, bk, Wv, Wo):
    xT = np.ascontiguousarray(x.reshape(T, D).T)
    scale = np.float32(1.0 / np.sqrt(DH))
    in_maps = []
    for c in range(NC):
        sl = slice(128 * c, 128 * (c + 1))
        in_maps.append({
            "xT": xT,
            "wq": np.ascontiguousarray((scale * Wq[sl, :]).T),
            "wk": np.ascontiguousarray(Wk[sl, :].T),
            "wv": np.ascontiguousarray(Wv[sl, :].T),
            "wo": np.ascontiguousarray(Wo[:, sl].T),
            "bq": np.ascontiguousarray((scale * bq[sl])[:, None]),
            "bk": np.ascontiguousarray(bk[sl][:, None]),
        })
    return in_maps


def kernel(x, Wq, bq, Wk, bk, Wv, bv, Wo, bo):
    x = np.asarray(x, np.float32)
    Wq, bq = np.asarray(Wq, np.float32), np.asarray(bq, np.float32)
    Wk, bk = np.asarray(Wk, np.float32), np.asarray(bk, np.float32)
    Wv, bv = np.asarray(Wv, np.float32), np.asarray(bv, np.float32)
    Wo, bo = np.asarray(Wo, np.float32), np.asarray(bo, np.float32)

    nc = _build()
    in_maps = _prep_in_maps(x, Wq, bq, Wk, bk, Wv, Wo)
    res = bass_utils.run_bass_kernel_spmd(nc, in_maps, core_ids=list(range(NC)))

    acc = np.zeros((D, T), np.float64)
    for c in range(NC):
        acc += res.results[c]["outT"]
    # v-bias folds through softmax (rows sum to 1): + bv @ Wo.T; plus bo.
    const = bo.astype(np.float64) + bv.astype(np.float64) @ Wo.T.astype(np.float64)
    out = acc.T + const[None, :]
    return out.astype(np.float32).reshape(B, S, D)


# revision 10
# speedup vs baseline: 1.3704x; 1.1172x over previous
import sys

if "/opt/trn_rl_repo" not in sys.path:
    sys.path.insert(0, "/opt/trn_rl_repo")

import numpy as np

import concourse.bacc as bacc
import concourse.tile as tile
from concourse import bass_utils, mybir
from concourse.bass import ts
from concourse.masks import make_identity

F32 = mybir.dt.float32
BF16 = mybir.dt.bfloat16
EXP = mybir.ActivationFunctionType.Exp

# nn_MultiHeadedAttention: B=2, S=2048, D=1024, H=16, DH=64.
# 16 heads over 8 cores (2 heads/core = 128 features). QKV column-parallel,
# out-projection row-parallel, host sums the 8 partial outputs.
B, S, D, H = 2, 2048, 1024, 16
DH = D // H
NC = 8
T = B * S                  # 4096 tokens
NCHUNK = T // 512          # 8 token chunks of 512
KCH = D // 128             # 8 contraction chunks
NJ = S // 128              # 16 key tiles per batch
QC = S // 512              # 4 query chunks per batch

_CACHE = {}


def _build():
    if "nc" in _CACHE:
        return _CACHE["nc"]

    nc = bacc.Bacc("TRN2", target_bir_lowering=False, debug=False,
                   enable_asserts=True, num_devices=NC)

    xT = nc.dram_tensor("xT", [D, T], BF16, kind="ExternalInput").ap()
    wq = nc.dram_tensor("wq", [D, 128], BF16, kind="ExternalInput").ap()
    wk = nc.dram_tensor("wk", [D, 128], BF16, kind="ExternalInput").ap()
    wv = nc.dram_tensor("wv", [D, 128], BF16, kind="ExternalInput").ap()
    wo = nc.dram_tensor("wo", [128, D], BF16, kind="ExternalInput").ap()
    bq = nc.dram_tensor("bq", [128, 1], F32, kind="ExternalInput").ap()
    bk = nc.dram_tensor("bk", [128, 1], F32, kind="ExternalInput").ap()
    outT = nc.dram_tensor("outT", [D, T], F32, kind="ExternalOutput").ap()

    with tile.TileContext(nc) as tc:
        with (
            tc.tile_pool(name="wpool", bufs=1) as wpool,
            tc.tile_pool(name="qk", bufs=1) as qk_pool,
            tc.tile_pool(name="vtm", bufs=1) as vtm_pool,
            tc.tile_pool(name="on", bufs=1) as on_pool,
            tc.tile_pool(name="xin", bufs=12) as xin_pool,
            tc.tile_pool(name="vst", bufs=2) as vst_pool,
            tc.tile_pool(name="epool", bufs=6) as epool,
            tc.tile_pool(name="npool", bufs=2) as npool,
            tc.tile_pool(name="ostage", bufs=3) as ostage_pool,
            # PSUM: psA 4 banks (qkv accum / scores), psO 2 (o accum),
            # psOP 2 (v transposes / out-projection)
            tc.tile_pool(name="psA", bufs=2, space="PSUM") as psA,
            tc.tile_pool(name="psO", bufs=1, space="PSUM") as psO,
            tc.tile_pool(name="psOP", bufs=2, space="PSUM") as psOP,
        ):
            # ---- persistent weights / constants ----
            wq_sb = wpool.tile([128, D], BF16)
            wk_sb = wpool.tile([128, D], BF16)
            wv_sb = wpool.tile([128, D], BF16)
            wo_sb = wpool.tile([128, D], BF16)
            bq_sb = wpool.tile([128, 1], F32)
            bk_sb = wpool.tile([128, 1], F32)
            ident = wpool.tile([128, 128], F32)
            for k in range(KCH):
                nc.sync.dma_start(wq_sb[:, ts(k, 128)], wq[ts(k, 128), :])
                nc.sync.dma_start(wk_sb[:, ts(k, 128)], wk[ts(k, 128), :])
                nc.sync.dma_start(wv_sb[:, ts(k, 128)], wv[ts(k, 128), :])
            nc.sync.dma_start(wo_sb[:], wo[:])
            nc.sync.dma_start(bq_sb[:], bq[:])
            nc.sync.dma_start(bk_sb[:], bk[:])
            make_identity(nc, ident[:])

            # Warm the ACT exp table while phase 1 runs.
            dummy = wpool.tile([1, 2], F32)
            nc.vector.memset(dummy[:], 0.0)
            nc.scalar.activation(dummy[:], dummy[:], EXP)

            # persistent activations
            qn = [qk_pool.tile([128, 512], BF16, name=f"qn{n}")
                  for n in range(NCHUNK)]
            kn = [qk_pool.tile([128, 512], BF16, name=f"kn{n}")
                  for n in range(NCHUNK)]
            on = [on_pool.tile([128, 512], BF16, name=f"on{n}")
                  for n in range(NCHUNK)]
            v_tm = {}
            for hh in range(2):
                for J in range(2 * NJ):
                    v_tm[(hh, J)] = vtm_pool.tile(
                        [128, 65], BF16, name=f"vtm{hh}_{J}")

            # ---- phase 1: projections + v transposes ----
            for n in range(NCHUNK):
                xps = []
                for k in range(KCH):
                    xp = xin_pool.tile([128, 512], BF16, tag="xp",
                                       name=f"xp{n}_{k}")
                    nc.sync.dma_start(xp[:], xT[ts(k, 128), ts(n, 512)])
                    xps.append(xp)
                qk_ps = psA.tile([128, 1024], F32, tag="A", name=f"qkps{n}")
                v_ps = psA.tile([128, 1024], F32, tag="A", name=f"vps{n}")
                for k in range(KCH):
                    nc.tensor.matmul(qk_ps[:, 0:512], wq_sb[:, ts(k, 128)],
                                     xps[k][:], start=(k == 0),
                                     stop=(k == KCH - 1))
                for k in range(KCH):
                    nc.tensor.matmul(qk_ps[:, 512:1024], wk_sb[:, ts(k, 128)],
                                     xps[k][:], start=(k == 0),
                                     stop=(k == KCH - 1))
                nc.vector.tensor_scalar_add(qn[n][:], qk_ps[:, 0:512],
                                            bq_sb[:])
                for k in range(KCH):
                    nc.tensor.matmul(v_ps[:, 0:512], wv_sb[:, ts(k, 128)],
                                     xps[k][:], start=(k == 0),
                                     stop=(k == KCH - 1))
                nc.vector.tensor_scalar_add(kn[n][:], qk_ps[:, 512:1024],
                                            bk_sb[:])
                vst = vst_pool.tile([128, 512], F32, name=f"vst{n}")
                nc.vector.tensor_copy(vst[:], v_ps[:, 0:512])
                for hh in range(2):
                    hs = slice(hh * 64, (hh + 1) * 64)
                    for jj in range(4):
                        J = 4 * n + jj
                        t_ps = psOP.tile([128, 512], F32, tag="OP",
                                         name=f"tps{n}_{hh}_{jj}")
                        nc.tensor.transpose(t_ps[:, 0:64],
                                            vst[hs, ts(jj, 128)],
                                            ident[hs, hs])
                        vt = v_tm[(hh, J)]
                        nc.vector.tensor_copy(vt[:, 0:64], t_ps[:, 0:64])
                        nc.vector.memset(vt[:, 64:65], 1.0)

            # ---- phase 2: attention + out-projection ----
            # Normalize + out-projection for a finished query chunk is
            # emitted a few j-iterations into the NEXT chunk so the PE
            # never waits on the normalization chain.
            def emit_norm_outproj(n, o_ps):
                oc = npool.tile([64, 1024], F32, tag="oc", name=f"oc{n}")
                nc.vector.tensor_copy(oc[:], o_ps[0:64, :])
                # sums row lives at partition 64; reciprocal_approx_fast
                # mishandles base_partition != 0, so shift to partition 0.
                sums_sb = npool.tile([1, 1024], F32, tag="sums",
                                     name=f"sums{n}")
                nc.vector.tensor_copy(sums_sb[:], o_ps[64:65, :])
                r_sb = npool.tile([1, 1024], F32, tag="r", name=f"r{n}")
                nc.vector.reciprocal_approx_fast(r_sb[:], sums_sb[0:1, :])
                for hh in range(2):
                    hs = slice(hh * 64, (hh + 1) * 64)
                    rb = npool.tile([64, 512], F32, tag=f"rb{hh}",
                                    name=f"rb{n}_{hh}")
                    nc.gpsimd.partition_broadcast(
                        rb[:], r_sb[0:1, ts(hh, 512)])
                    nc.vector.tensor_tensor(
                        out=on[n][hs, :], in0=oc[0:64, ts(hh, 512)],
                        in1=rb[:], op=mybir.AluOpType.mult)
                for m in range(KCH):
                    op_ps = psOP.tile([128, 512], F32, tag="OP",
                                      name=f"opps{n}_{m}")
                    nc.tensor.matmul(op_ps[:, 0:512], wo_sb[:, ts(m, 128)],
                                     on[n][:], start=True, stop=True)
                    ost = ostage_pool.tile([128, 512], F32, tag="ost",
                                           name=f"ost{n}_{m}")
                    nc.vector.tensor_copy(ost[:], op_ps[:, 0:512])
                    nc.sync.dma_start(outT[ts(m, 128), ts(n, 512)], ost[:])

            pending = None
            for b in range(B):
                for qc in range(QC):
                    n = b * QC + qc
                    o_ps = psO.tile([65, 1024], F32, tag="O", name=f"ops{n}")
                    e_prev = None
                    for j in range(NJ):
                        s_ps = psA.tile([128, 1024], F32, tag="A",
                                        name=f"sps{n}_{j}")
                        for hh in range(2):
                            hs = slice(hh * 64, (hh + 1) * 64)
                            nc.tensor.matmul(
                                s_ps[:, ts(hh, 512)],
                                kn[b * QC + j // 4][hs, ts(j % 4, 128)],
                                qn[n][hs, :], start=True, stop=True)
                        e_sb = epool.tile([128, 1024], BF16, tag="e",
                                          name=f"e{n}_{j}")
                        nc.scalar.activation(e_sb[:], s_ps[:], EXP)
                        if j >= 1:
                            for hh in range(2):
                                nc.tensor.matmul(
                                    o_ps[0:65, ts(hh, 512)],
                                    v_tm[(hh, b * NJ + j - 1)][:],
                                    e_prev[:, ts(hh, 512)],
                                    start=(j - 1 == 0), stop=False)
                        if j == 4 and pending is not None:
                            emit_norm_outproj(*pending)
                            pending = None
                        e_prev = e_sb
                    for hh in range(2):
                        nc.tensor.matmul(
                            o_ps[0:65, ts(hh, 512)],
                            v_tm[(hh, b * NJ + NJ - 1)][:],
                            e_prev[:, ts(hh, 512)],
                            start=False, stop=True)
                    pending = (n, o_ps)
            emit_norm_outproj(*pending)

    nc.compile()
    _CACHE["nc"] = nc
    return nc


def _prep_in_maps(x, Wq, bq, Wk, bk, Wv, Wo):
    import ml_dtypes
    bf16 = ml_dtypes.bfloat16
    xT = np.ascontiguousarray(x.reshape(T, D).T).astype(bf16)
    scale = np.float32(1.0 / np.sqrt(DH))
    in_maps = []
    for c in range(NC):
        sl = slice(128 * c, 128 * (c + 1))
        in_maps.append({
            "xT": xT,
            "wq": np.ascontiguousarray((scale * Wq[sl, :]).T).astype(bf16),
            "wk": np.ascontiguousarray(Wk[sl, :].T).astype(bf16),
            "wv": np.ascontiguousarray(Wv[sl, :].T).astype(bf16),
            "wo": np.ascontiguousarray(Wo[:, sl].T).astype(bf16),
            "bq": np.ascontiguousarray((scale * bq[sl])[:, None]),
            "bk": np.ascontiguousarray(bk[sl][:, None]),
        })
    return in_maps


def kernel(x, Wq, bq, Wk, bk, Wv, bv, Wo, bo):
    x = np.asarray(x, np.float32)
    Wq, bq = np.asarray(Wq, np.float32), np.asarray(bq, np.float32)
    Wk, bk = np.asarray(Wk, np.float32), np.asarray(bk, np.float32)
    Wv, bv = np.asarray(Wv, np.float32), np.asarray(bv, np.float32)
    Wo, bo = np.asarray(Wo, np.float32), np.asarray(bo, np.float32)

    nc = _build()
    in_maps = _prep_in_maps(x, Wq, bq, Wk, bk, Wv, Wo)
    res = bass_utils.run_bass_kernel_spmd(nc, in_maps, core_ids=list(range(NC)))

    acc = np.zeros((D, T), np.float64)
    for c in range(NC):
        acc += res.results[c]["outT"]
    # v-bias folds through softmax (rows sum to 1): + bv @ Wo.T; plus bo.
    const = bo.astype(np.float64) + bv.astype(np.float64) @ Wo.T.astype(np.float64)
    out = acc.T + const[None, :]
    return out.astype(np.float32).reshape(B, S, D)
